# revision 17
# baseline (speedup 1.0000x reference)
"""Trainium2 Bass kernel for nn_CrossModalFusionCore (B=8, S=1024, D=1024, H=16).

Structure exploited: in the reference, K/V of the first cross-attention come
from a per-batch vector broadcast across the sequence (softmax over identical
scores -> uniform -> output == V vector), and the queries of the second
cross-attention are all identical (one attention distribution per head per
batch). Hence the entire output is constant across the sequence dimension,
and per batch the real tensor work is:

  scores[s,h] = (seq_b[s] . M_b[:,h] + c_b[h]) / 8   (M_b = Wk_h^T q_h)
  attn = softmax_s(scores);  w_b = seq_b^T @ attn                [D,H]
  ctx[h*64+j] = Wv_h[j] . w_b[:,h] + bv   (folded into gl0/pl0/sa0 consts)
  u1 = ow @ ctx; u2 = G2 @ ctx; u3 = P2 @ ctx   (G2=gw[:,D:]@ow, P2=pw[:,D:]@ow)
  gate = sigmoid(u2 + gl0);  x = pl0p + u3 + u1 + gate*(sa0 - u1)
  out_b[s,:] = LayerNorm(x) for all s

Distribution: PURE data-parallel over batch -- core b owns batch b end to
end, ZERO collectives. The previous tensor-parallel epilogue spent most of
the wall clock in an AllToAll (18us + 22us trigger delay) and an AllReduce
(10us) plus cross-core skew coupling; the whole epilogue is only ~5M MACs,
so each core instead loads the full (host-composed) weight matrices
(~8MB bf16, overlapped with the attention phase) and runs the epilogue as
vector-in-array matmuls: the per-batch vector is the 1-column stationary
operand (LDWEIGHTS ~= free) and the weight matrix streams through as rhs at
N=512. Output is written as bf16 (host upcasts to f32).
"""
import numpy as np
import ml_dtypes
from contextlib import ExitStack

import concourse.bass as bass
import concourse.tile as tile
from concourse import bacc, mybir
from concourse.bass_utils import run_bass_kernel_spmd
from concourse.masks import make_identity

B, S, D, H = 8, 1024, 1024, 16
HD = D // H
NCORES = 8
EPS = 1e-5
BF = mybir.dt.bfloat16
F32 = mybir.dt.float32
F8 = mybir.dt.float8e4
ASC = 64.0  # attn scale: keeps fp8 operands in normal range; LN cancels it

# test.py hooks
TRACE = False
TRACE_CORES = None
LAST_RESULT = None

_cache = {}


def _body(ctx, tc, io):
    nc = tc.nc
    const = ctx.enter_context(tc.tile_pool(name="const", bufs=1))
    work = ctx.enter_context(tc.tile_pool(name="work", bufs=1))
    psum = ctx.enter_context(tc.tile_pool(name="psum", bufs=3, space="PSUM"))

    def ps_mm(name):
        return psum.tile([128, 512], F32, tag="mm", bufs=4, name=name)

    def ps_tp(name, dt=BF):
        return psum.tile([128, 512], dt, tag="tp", bufs=2, name=name)

    # ---- tiny constants ----
    ident = const.tile([128, 128], BF)
    make_identity(nc, ident)
    identf = const.tile([128, 128], F32)
    make_identity(nc, identf)
    ones_col = const.tile([128, 1], F32)
    nc.vector.memset(ones_col[:, :], 1.0)
    ones_row_f = const.tile([1, 128], F32)
    nc.vector.memset(ones_row_f[:, :], 1.0)
    ones_row_b = const.tile([1, 128], BF)
    nc.vector.memset(ones_row_b[:, :], 1.0)
    epst = const.tile([1, 1], F32)
    nc.vector.memset(epst[:, :], EPS * ASC * ASC)
    c64 = const.tile([H, 1], F32)
    nc.vector.memset(c64[:, :], ASC)

    # PE warm-up: ~3.4us of dummy transposes flips the HAM clock gate to
    # 2.4GHz before the first real matmul (otherwise scores run at 1.2GHz)
    ps_warm = psum.tile([128, 512], BF, tag="bc", bufs=2, name="warm")
    for i in range(32):
        nc.tensor.transpose(ps_warm[:, (i % 4) * 128:(i % 4 + 1) * 128],
                            ident[:, :], ident[:, :])

    # ---- loads: queue A = sync, queue B = scalar ----
    msc_sb = const.tile([128, 8, H], F8)
    nc.scalar.dma_start(out=msc_sb[:, :, :], in_=io["msc"])
    cb8_sb = const.tile([H, 1], F32)
    nc.scalar.dma_start(out=cb8_sb[:, :], in_=io["cb8"])
    vec5_sb = const.tile([128, 5, 8], F32)
    nc.scalar.dma_start(out=vec5_sb[:, :, :], in_=io["vec5"])

    seqT_sb = const.tile([128, 8, S], F8)  # [d%128, d//128, s]
    nc.sync.dma_start(out=seqT_sb[:, 0:4, :], in_=io["seqT"][:, 0:4, :])
    nc.sync.dma_start(out=seqT_sb[:, 4:8, :], in_=io["seqT"][:, 4:8, :])
    seqN_sb = const.tile([128, 8, D], F8)  # [s%128, s//128, d]
    nc.scalar.dma_start(out=seqN_sb[:, :, :], in_=io["seqN"])

    wvT_sb = const.tile([128, 8, D], F8)   # [d%128, d//128, ctx-out j]
    nc.scalar.dma_start(out=wvT_sb[:, :, :], in_=io["wvT"])
    w3_sb = const.tile([128, 8, 3, D], F8)  # [ctx%128, ctx//128, {ow,G2,P2}, out j]
    nc.sync.dma_start(out=w3_sb[:, 0:4, :, :], in_=io["w3c"][:, 0:4, :, :])
    nc.scalar.dma_start(out=w3_sb[:, 4:8, :, :], in_=io["w3c"][:, 4:8, :, :])

    # ---- phase 1: scores^T = M^T @ seq^T, exp((scores+c)/8) fused on ACT ----
    scope_p1 = nc.named_scope("p1_attn"); scope_p1.__enter__()
    expT = work.tile([H, S], F32)
    ps_sc = [ps_mm("ps0")[0:H, :], ps_mm("ps1")[0:H, :]]
    for c in range(8):
        for half in range(2):
            nc.tensor.matmul(ps_sc[half][:, :], msc_sb[:, c, :],
                             seqT_sb[:, c, 512 * half:512 * (half + 1)],
                             start=(c == 0), stop=(c == 7))
    for half in range(2):
        nc.scalar.activation(out=expT[:, 512 * half:512 * (half + 1)],
                             in_=ps_sc[half][:, :],
                             func=mybir.ActivationFunctionType.Exp,
                             bias=cb8_sb[:, :], scale=0.125)

    ssum = work.tile([H, 1], F32)
    nc.vector.reduce_sum(out=ssum[:, :], in_=expT[:, :], axis=mybir.AxisListType.X)
    # keep-alive: PE op gated on ssum fills the softmax gap so the HAM
    # activity window stays busy and the clock stays at 2.4GHz
    ka_ps = ps_tp("ka", F32)[0:1, 0:H]
    nc.tensor.transpose(ka_ps[:, :], ssum[:, :], identf[0:H, 0:H])
    rsum = work.tile([H, 1], F32)
    nc.vector.reciprocal(out=rsum[:, :], in_=ssum[:, :])
    attnT = work.tile([H, S], BF)
    nc.vector.tensor_scalar(out=attnT[:, :], in0=expT[:, :],
                            scalar1=rsum[:, :], scalar2=c64[:, :],
                            op0=mybir.AluOpType.mult,
                            op1=mybir.AluOpType.mult)

    # transpose attn to [s%128, (s//128, h)]
    tpa = ps_tp("tpa")[:, 0:128]
    for c in range(8):
        nc.tensor.transpose(tpa[:, c * H:(c + 1) * H],
                            attnT[:, c * 128:(c + 1) * 128], ident[0:H, 0:H])
    attn_sb = work.tile([128, 128], F8)
    nc.vector.tensor_copy(out=attn_sb[:, :], in_=tpa[:, :])

    # w^T = attn^T @ seq -> [H, D]
    wT = work.tile([H, D], BF)
    ps_w = [ps_mm("psw0")[0:H, :], ps_mm("psw1")[0:H, :]]
    for c in range(8):
        for half in range(2):
            nc.tensor.matmul(ps_w[half][:, :], attn_sb[:, c * H:(c + 1) * H],
                             seqN_sb[:, c, 512 * half:512 * (half + 1)],
                             start=(c == 0), stop=(c == 7))
    nc.vector.tensor_copy(out=wT[:, 0:512], in_=ps_w[0][:, :])
    nc.scalar.mul(out=wT[:, 512:1024], in_=ps_w[1][:, :], mul=1.0)

    # transpose w to wD [d%128, (d//128, h)]
    tpw = ps_tp("tpw")[:, 0:128]
    for c in range(8):
        nc.tensor.transpose(tpw[:, c * H:(c + 1) * H],
                            wT[:, c * 128:(c + 1) * 128], ident[0:H, 0:H])
    wD = work.tile([128, 128], F8)
    nc.vector.tensor_copy(out=wD[:, :], in_=tpw[:, :])
    scope_p1.__exit__(None, None, None)

    # ---- phase 2: ctx diagonal. ctx[j] = wv[j] . w[j//64] ----
    # one matmul per (head, d-chunk): lhsT = w_h chunk column [128,1],
    # rhs = wv^T block for head h's 64 outputs -> psum row 0
    scope_p3 = nc.named_scope("p3_ctx"); scope_p3.__enter__()
    cr_ps = [ps_mm("cr0")[0:1, :], ps_mm("cr1")[0:1, :]]
    for c in range(8):
        for h in range(H):
            dst = cr_ps[h // 8][:, (h % 8) * 64:(h % 8) * 64 + 64]
            nc.tensor.matmul(dst, wD[:, c * H + h:c * H + h + 1],
                             wvT_sb[:, c, h * 64:(h + 1) * 64],
                             start=(c == 0), stop=(c == 7))
    ctx_row = work.tile([1, D], F32)
    nc.vector.tensor_copy(out=ctx_row[:, 0:512], in_=cr_ps[0][:, :])
    nc.scalar.mul(out=ctx_row[:, 512:1024], in_=cr_ps[1][:, :], mul=1.0)

    # to partition layout: ctx_sb[p, c] = ctx[c*128+p], 8 transposes
    ctxT_ps = ps_tp("ctxT", F32)[:, 0:8]
    for c in range(8):
        nc.tensor.transpose(ctxT_ps[:, c:c + 1],
                            ctx_row[0:1, c * 128:(c + 1) * 128],
                            identf[0:1, 0:1])
    ctx_sb = work.tile([128, 8], F8)
    nc.vector.tensor_copy(out=ctx_sb[:, :], in_=ctxT_ps[:, :])
    scope_p3.__exit__(None, None, None)

    # ---- phase 3: projections u = [ow; G2; P2] @ ctx as [1,512] rows ----
    # psum rows at 32*i so the fixup transposes read from legal base
    # partitions (PE inputs must start at partition 0/32/64)
    scope_p5 = nc.named_scope("p5_proj"); scope_p5.__enter__()
    u_ps = [ps_mm("uA")[0:65, :], ps_mm("uB")[0:65, :]]
    for c in range(8):
        for i in range(3):
            for hf in range(2):
                nc.tensor.matmul(u_ps[hf][32 * i:32 * i + 1, :],
                                 ctx_sb[:, c:c + 1],
                                 w3_sb[:, c, i, 512 * hf:512 * (hf + 1)],
                                 start=(c == 0), stop=(c == 7))
    u_sbh = []
    for hf in range(2):
        t = work.tile([65, 512], F32, name=f"u_sbh{hf}")
        if hf == 0:
            nc.vector.tensor_copy(out=t[:, :], in_=u_ps[0][:, :])
        else:
            nc.scalar.mul(out=t[:, :], in_=u_ps[1][:, :], mul=1.0)
        u_sbh.append(t)

    # fix layout: u_sb[p, i*8 + hf*4 + blk] = u_i[(hf*4+blk)*128 + p]
    fix_ps = ps_tp("fix", F32)[:, 0:24]
    for i in range(3):
        for hf in range(2):
            for blk in range(4):
                col = i * 8 + hf * 4 + blk
                nc.tensor.transpose(
                    fix_ps[:, col:col + 1],
                    u_sbh[hf][32 * i:32 * i + 1, blk * 128:(blk + 1) * 128],
                    identf[32 * i:32 * i + 1, 32 * i:32 * i + 1])
    u_sb = work.tile([128, 24], F32)
    nc.vector.tensor_copy(out=u_sb[:, :], in_=fix_ps[:, :])
    u1 = u_sb[:, 0:8]
    u2 = u_sb[:, 8:16]
    u3 = u_sb[:, 16:24]
    scope_p5.__exit__(None, None, None)

    # ---- phase 4: gate/fuse/LayerNorm tail on [128, 8] tiles ----
    scope_p6 = nc.named_scope("p6_tail"); scope_p6.__enter__()
    gl = work.tile([128, 8], F32)
    nc.vector.tensor_add(out=gl[:, :], in0=u2, in1=vec5_sb[:, 1, :])
    gate = work.tile([128, 8], F32)
    nc.scalar.activation(out=gate[:, :], in_=gl[:, :],
                         func=mybir.ActivationFunctionType.Sigmoid,
                         scale=1.0 / ASC)
    d1 = work.tile([128, 8], F32)
    nc.vector.tensor_sub(out=d1[:, :], in0=vec5_sb[:, 0, :], in1=u1)
    gd = work.tile([128, 8], F32)
    nc.vector.tensor_mul(out=gd[:, :], in0=gate[:, :], in1=d1[:, :])
    t1 = work.tile([128, 8], F32)
    nc.vector.tensor_add(out=t1[:, :], in0=u1, in1=u3)
    t2 = work.tile([128, 8], F32)
    nc.vector.tensor_add(out=t2[:, :], in0=t1[:, :], in1=vec5_sb[:, 2, :])
    xx = work.tile([128, 16], F32)
    nc.vector.tensor_add(out=xx[:, 0:8], in0=t2[:, :], in1=gd[:, :])
    nc.vector.tensor_mul(out=xx[:, 8:16], in0=xx[:, 0:8], in1=xx[:, 0:8])

    sums_ps = ps_tp("sums", F32)[0:1, 0:16]
    nc.tensor.matmul(sums_ps[:, :], ones_col[:, :], xx[:, :],
                     start=True, stop=True)
    s0 = work.tile([1, 2], F32)
    nc.vector.reduce_sum(out=s0[:, 0:1], in_=sums_ps[:, 0:8],
                         axis=mybir.AxisListType.X)
    nc.vector.reduce_sum(out=s0[:, 1:2], in_=sums_ps[:, 8:16],
                         axis=mybir.AxisListType.X)
    m2 = work.tile([1, 2], F32)   # [mu, ex2] then [mu, rstd]
    nc.scalar.mul(out=m2[:, :], in_=s0[:, :], mul=1.0 / D)
    musq = work.tile([1, 1], F32)
    nc.vector.tensor_mul(out=musq[:, :], in0=m2[:, 0:1], in1=m2[:, 0:1])
    varv = work.tile([1, 1], F32)
    nc.vector.tensor_sub(out=varv[:, :], in0=m2[:, 1:2], in1=musq[:, :])
    sd = work.tile([1, 1], F32)
    nc.scalar.activation(out=sd[:, :], in_=varv[:, :],
                         func=mybir.ActivationFunctionType.Sqrt,
                         bias=epst[:, :])
    nc.vector.reciprocal(out=m2[:, 1:2], in_=sd[:, :])
    mr_ps = ps_tp("mr", F32)[0:128, 0:2]
    nc.tensor.matmul(mr_ps[:, :], ones_row_f[:, :], m2[:, :],
                     start=True, stop=True)
    mr128 = work.tile([128, 2], F32)
    nc.vector.tensor_copy(out=mr128[:, :], in_=mr_ps[:, :])

    xn = work.tile([128, 8], F32)
    nc.vector.tensor_scalar(out=xn[:, :], in0=xx[:, 0:8],
                            scalar1=mr128[:, 0:1], scalar2=mr128[:, 1:2],
                            op0=mybir.AluOpType.subtract,
                            op1=mybir.AluOpType.mult)
    yg = work.tile([128, 8], F32)
    nc.vector.tensor_mul(out=yg[:, :], in0=xn[:, :], in1=vec5_sb[:, 3, :])
    y_bf = work.tile([128, 8], BF)
    nc.vector.tensor_add(out=y_bf[:, :], in0=yg[:, :], in1=vec5_sb[:, 4, :])

    # broadcast y across partitions: y -> row [1, 1024] (8 column
    # transposes, all base partition 0), then ones-outer-product matmuls
    yrow_ps = ps_tp("yrow")[0:1, :]
    yrow_ps2 = psum.tile([128, 512], BF, tag="bc", bufs=2, name="yrow2")[0:1, :]
    for c in range(8):
        dst = (yrow_ps if c < 4 else yrow_ps2)[:, (c % 4) * 128:(c % 4 + 1) * 128]
        nc.tensor.transpose(dst, y_bf[:, c:c + 1], ident[:, :])
    y_row = work.tile([1, D], BF)
    nc.vector.tensor_copy(out=y_row[:, 0:512], in_=yrow_ps[:, :])
    nc.scalar.mul(out=y_row[:, 512:1024], in_=yrow_ps2[:, :], mul=1.0)
    # ybig holds 4 replicas of the broadcast row block so each partition
    # carries 4 consecutive output rows -> 8KB DMA descriptors
    ybig = work.tile([128, 4096], BF)
    bc_ps = [psum.tile([128, 512], F32, tag="bc", bufs=2, name=f"bc{h}")
             for h in range(2)]
    for half in range(2):
        nc.tensor.matmul(bc_ps[half][:, :], ones_row_b[:, :],
                         y_row[0:1, 512 * half:512 * (half + 1)],
                         start=True, stop=True)
    for r in range(4):
        for half in range(2):
            dst = ybig[:, r * 1024 + half * 512:r * 1024 + half * 512 + 512]
            if (r * 2 + half) % 2 == 0:
                nc.vector.tensor_copy(out=dst, in_=bc_ps[half][:, :])
            else:
                nc.scalar.mul(out=dst, in_=bc_ps[half][:, :], mul=1.0)
    scope_p6.__exit__(None, None, None)

    # ---- write out [S, D] bf16: 2 DMAs, each [128, 4096] -> 512 rows ----
    scope_p7 = nc.named_scope("p7_write"); scope_p7.__enter__()
    def out_slice(k):
        a = io["out"]
        return bass.AP(tensor=a.tensor, offset=k * 512 * 1024,
                       ap=[[4096, 128], [1, 4096]])
    nc.sync.dma_start(out=out_slice(0), in_=ybig[:, :])
    nc.scalar.dma_start(out=out_slice(1), in_=ybig[:, :])
    scope_p7.__exit__(None, None, None)


def _build():
    if "nc" in _cache:
        return _cache["nc"]
    nc = bacc.Bacc("TRN2", target_bir_lowering=False, debug=False,
                   enable_asserts=False, num_devices=NCORES)
    io = {}

    def inp(name, shape, dt):
        io[name] = nc.dram_tensor(name, shape, dt, kind="ExternalInput").ap()

    inp("seqT", [128, 8, S], F8)
    inp("seqN", [128, 8, D], F8)
    inp("msc", [128, 8, H], F8)
    inp("cb8", [H, 1], F32)
    inp("wvT", [128, 8, D], F8)
    inp("w3c", [128, 8, 3, D], F8)
    inp("vec5", [128, 5, 8], F32)
    io["out"] = nc.dram_tensor("out", [S, D], BF, kind="ExternalOutput").ap()

    with tile.TileContext(nc) as tc:
        with ExitStack() as ctx:
            _body(ctx, tc, io)
    nc.compile()
    _cache["nc"] = nc
    return nc


def _host_prep(inputs):
    seq = np.asarray(inputs["seq_repr"], np.float32)
    g = np.asarray(inputs["graph_repr"], np.float32)
    ipw = np.asarray(inputs["in_proj_w"], np.float32)
    ipb = np.asarray(inputs["in_proj_b"], np.float32)
    ow = np.asarray(inputs["out_w"], np.float32)
    ob = np.asarray(inputs["out_b"], np.float32)
    gw = np.asarray(inputs["gate_w"], np.float32)
    gb = np.asarray(inputs["gate_b"], np.float32)
    pw = np.asarray(inputs["proj_w"], np.float32)
    pb = np.asarray(inputs["proj_b"], np.float32)
    ln_g = np.asarray(inputs["ln_g"], np.float32)
    ln_b = np.asarray(inputs["ln_b"], np.float32)

    wq, wk, wv = ipw[:D], ipw[D:2 * D], ipw[2 * D:]
    bq, bk, bv = ipb[:D], ipb[D:2 * D], ipb[2 * D:]

    q_g = g @ wq.T + bq                      # [B, D]
    v_g = g @ wv.T + bv                      # [B, D]
    qh = q_g.reshape(B, H, HD)
    M = np.einsum("bhr,hrd->bdh", qh, wk.reshape(H, HD, D))  # [B, D, H]
    c = np.einsum("bhr,hr->bh", qh, bk.reshape(H, HD))       # [B, H]
    sa = v_g @ ow.T + ob                     # [B, D]
    G1 = gw[:, :D] @ ow
    G2 = gw[:, D:] @ ow
    P1 = pw[:, :D] @ ow
    P2 = pw[:, D:] @ ow
    gtb = (gw[:, :D] + gw[:, D:]) @ ob + gb
    ptb = (pw[:, :D] + pw[:, D:]) @ ob + pb
    # bv folded: ctx on device omits +bv, so fold bv's contribution of
    # u_i = W_i @ (ctx + bv_vec) into the host constants.
    bvv = bv                                  # [D] ctx bias vector
    gl0 = v_g @ G1.T + gtb + G2 @ bvv        # [B, D]
    pl0 = v_g @ P1.T + ptb + P2 @ bvv        # [B, D]
    sa0 = sa - ob - ow @ bvv                 # [B, D] (sa0 - u1 needs true ga)
    pl0p = pl0 + ob + ow @ bvv               # ob + ow@bv folded into x's sum
    # NOTE: x = pl0p + u3 + u1 + gate*(sa0 - u1) where u1 = ow@ctx_nobias.
    # True ga = ow@(ctx_nobias + bv) + ob = u1 + ow@bv + ob. Substituting:
    #   x = pl0 + ob + P2@bv_part... -- handled by the folds above:
    #   pl2_true + ga_true = u3 + u1 + (P2@bv) + (ow@bv + ob)  -> in pl0p/gl0
    #   gate arg: gl0 + G2@bv + u2; sa - ga_true = (sa - ow@bv - ob) - u1.

    bf = ml_dtypes.bfloat16
    f8 = ml_dtypes.float8_e4m3
    f32 = np.float32

    def tile128(a):  # [1024, N] -> [128, 8, N] with p = dim0 % 128
        n = a.shape[1]
        return np.ascontiguousarray(
            a.reshape(8, 128, n).transpose(1, 0, 2))

    wvT_t = tile128(wv.T).astype(f8)                       # [128, 8, 1024]
    w3 = np.stack([ow.T, G2.T, P2.T], axis=1)              # [1024, 3, 1024]
    w3c_t = np.ascontiguousarray(
        w3.reshape(8, 128, 3, D).transpose(1, 0, 2, 3)).astype(f8)

    in_maps = []
    for j in range(NCORES):
        vec5 = np.stack([64.0 * sa0[j], 64.0 * gl0[j], 64.0 * pl0p[j],
                         ln_g, ln_b], axis=0)              # [5, 1024]
        vec5 = np.ascontiguousarray(
            vec5.reshape(5, 8, 128).transpose(2, 0, 1))    # [128, 5, 8]
        in_maps.append({
            "seqT": tile128(np.ascontiguousarray(seq[j].T)).astype(f8),
            "seqN": tile128(seq[j]).astype(f8),
            "msc": np.ascontiguousarray(
                M[j].reshape(8, 128, H).transpose(1, 0, 2)).astype(f8),
            "cb8": (c[j] / 8.0).reshape(H, 1).astype(f32),
            "wvT": wvT_t,
            "w3c": w3c_t,
            "vec5": vec5.astype(f32),
        })
    return in_maps


def kernel(**inputs):
    global LAST_RESULT
    nc = _build()
    in_maps = _host_prep(inputs)
    kwargs = {}
    if TRACE:
        kwargs = dict(trace=True,
                      trace_cores=TRACE_CORES or list(range(NCORES)))
    res = run_bass_kernel_spmd(nc, in_maps, list(range(NCORES)), **kwargs)
    LAST_RESULT = res
    out = np.stack([np.asarray(res.results[j]["out"]).astype(np.float32)
                    for j in range(NCORES)], axis=0)
    return out
